# revision 7
# baseline (speedup 1.0000x reference)
"""Trainium2 Bass kernel for nn_Decoder_25718264168590.

2-layer LSTM decoder (B=32, T=50, H=1024, E=128) + vocab projection
(V=32000) + log_softmax, distributed over 8 NeuronCores.

v3 design:

- LSTM gate-sharded 8 ways (core r owns h-units [r*128,(r+1)*128) of
  both layers).  Recurrent matmuls are BATCH-MAJOR: out[batch=32,
  gates=512] with the h-tile [128,32] stationary and the weight tile
  [128,512] moving -- 26 long matmuls per tick instead of the 108 tiny
  ones a gate-major layout needs.
- The per-tick AllGather is SPLIT in two: AG0 ships h0(t) right after
  the layer-0 cell, AG1 ships h1(t-1) at the end of the tick.  whh1
  matmuls are emitted last inside the tick so the previous tick's AG1
  has landed by the time PE reaches them.  Layer 1 lags layer 0 by one
  tick.
- h vectors are produced batch-major [32,128]; a DVE 32x32 block
  StreamTranspose + a strided DMA rebuild the [128,32] exchange layout
  without touching the PE.
- The vocab projection is INTERLEAVED into the tick loop: every 4th
  tick, the 128 samples whose h1 just completed are pushed through the
  fp8 matmuls.  This fills the PE while the AllGathers are in flight,
  keeps the PE p-state hot, and spreads the 25 MB/core output DMA over
  the whole run instead of a tail phase.
- Vocab matmuls run in fp8e4 DoubleRow perf mode (2 k-tiles per
  instruction at 0.5 cycles/row): lin_w is host-quantized at scale
  2^11, h1 cast on-device at scale 2^7 (small strided cast per tick,
  off the critical path); the PSUM->SBUF descale is 2^-18.  These
  errors do not feed back into the recurrence.
- log_softmax: fused exp+accumulate per m-tile, chunked AllReduce for
  the cross-core sum, no max pass (logits are O(1) so exp cannot
  overflow).  lin_b is all-zero in this model; the bias matmul is only
  emitted if a nonzero bias is supplied (host-checked build flag).
"""

import sys

for _p in ("/opt/trn_rl_repo",):
    if _p not in sys.path:
        sys.path.insert(0, _p)

import numpy as np
import ml_dtypes

B, T, H, E, V = 32, 50, 1024, 128, 32000
NCORES = 8
VS = V // NCORES          # 4000 vocab cols per core
S = B * T                 # 1600 samples, t-major on device: s = t*32 + b
KT = H // 128             # 8 k-tiles of hidden
GPERM = (0, 1, 3, 2)      # torch gate order i,f,g,o -> our col order i,f,o,g
NMT = 13                  # sample m-tiles in vocab phase (12*128 + 64)
NCHK = 8                  # vocab col chunks per core (8 * 500)
CHUNK = VS // NCHK        # 500
AR_CHUNKS = ((0, 4), (4, 8), (8, 12), (12, 13))  # lse AllReduce chunking

SW = 2048.0               # fp8 weight scale (|w|<=0.1 -> <=204.8)
SH = 128.0                # fp8 h1 scale (|h|<=1 -> <=128)
DESCALE = 1.0 / (SW * SH)

BF16 = ml_dtypes.bfloat16
FP8 = ml_dtypes.float8_e4m3

_BUILD_CACHE = {}


def _host_prep(inputs):
    """Fold projections and lay out per-core device arrays."""
    enc = np.asarray(inputs["enc_output"], np.float32)       # (B, H)
    target = np.asarray(inputs["target"], np.float32)        # (B, T, E)
    proj_w = np.asarray(inputs["proj_w"], np.float32)        # (E, H+E)
    proj_b = np.asarray(inputs["proj_b"], np.float32)        # (E,)
    w_ih0 = np.asarray(inputs["w_ih0"], np.float32)          # (4H, E)
    w_hh0 = np.asarray(inputs["w_hh0"], np.float32)          # (4H, H)
    b0 = np.asarray(inputs["b_ih0"], np.float32) + np.asarray(inputs["b_hh0"], np.float32)
    w_ih1 = np.asarray(inputs["w_ih1"], np.float32)          # (4H, H)
    w_hh1 = np.asarray(inputs["w_hh1"], np.float32)          # (4H, H)
    b1 = np.asarray(inputs["b_ih1"], np.float32) + np.asarray(inputs["b_hh1"], np.float32)
    lin_w = np.asarray(inputs["lin_w"], np.float32)          # (V, H)
    lin_b = np.asarray(inputs["lin_b"], np.float32)          # (V,)

    P1 = proj_w[:, :E].T                                     # (E, E)
    P2 = proj_w[:, E:].T                                     # (H, E)
    A1 = P1 @ w_ih0.T                                        # (E, 4H) x-path fold
    genc = (enc @ P2 + proj_b) @ w_ih0.T                     # (B, 4H) enc-path fold

    # t-major input features: xt[e, t*32+b] = target[b, t, e]
    xt = np.ascontiguousarray(
        target.transpose(1, 0, 2).reshape(S, E).T).astype(BF16)          # (128, 1600)

    # exchange-layout encoder init: ench[p, k*32+b] = enc[b, k*128+p]
    ench = np.ascontiguousarray(
        enc.T.reshape(KT, 128, B).transpose(1, 0, 2).reshape(128, KT * B)
    ).astype(BF16)

    lin_wT = lin_w.T                                         # (H, V)
    ident = np.eye(B, dtype=BF16)
    use_linb = bool(np.any(lin_b != 0.0))

    in_maps = []
    for r in range(NCORES):
        rows = np.concatenate(
            [np.arange(128) + g * H + r * 128 for g in GPERM])           # 512 gate rows
        m = {}
        m["whh0t"] = np.ascontiguousarray(
            w_hh0[rows].T.reshape(KT, 128, 4 * 128)).astype(BF16)
        m["a1"] = np.ascontiguousarray(A1[:, rows]).astype(BF16)         # (128, 512)
        m["wih1t"] = np.ascontiguousarray(
            w_ih1[rows].T.reshape(KT, 128, 4 * 128)).astype(BF16)
        m["whh1t"] = np.ascontiguousarray(
            w_hh1[rows].T.reshape(KT, 128, 4 * 128)).astype(BF16)
        m["xt"] = xt
        # per-sample layer-0 bias (enc path + b0), fed into PSUM by an
        # identity matmul.
        m["gb0"] = np.ascontiguousarray(genc[:, rows] + b0[rows]).astype(BF16)
        m["gb1"] = np.ascontiguousarray(
            np.broadcast_to(b1[rows], (B, 512))).astype(BF16)
        m["ident"] = ident
        m["ench"] = ench
        m["cinit"] = np.ascontiguousarray(enc[:, r * 128:(r + 1) * 128])  # (32,128) f32
        lw = lin_wT[:, r * VS:(r + 1) * VS]                              # (H, 4000)
        m["linw8"] = np.ascontiguousarray(
            (lw.reshape(KT, 128, VS) * SW)).astype(FP8)
        if use_linb:
            m["linb"] = np.ascontiguousarray(
                lin_b[r * VS:(r + 1) * VS] / DESCALE
            ).astype(np.float32).astype(BF16).reshape(1, VS)
        in_maps.append(m)
    return in_maps, use_linb


def _build(reps=1, use_linb=False):
    import concourse.bass as bass
    import concourse.tile as tile
    from concourse import bacc, mybir
    from contextlib import ExitStack

    f32 = mybir.dt.float32
    bf16 = mybir.dt.bfloat16
    fp8 = mybir.dt.float8e4
    AF = mybir.ActivationFunctionType
    ALU = mybir.AluOpType
    DR = mybir.MatmulPerfMode.DoubleRow

    nc = bacc.Bacc("TRN2", target_bir_lowering=False, debug=False,
                   num_devices=NCORES)

    d_whh0 = nc.dram_tensor("whh0t", [KT, 128, 512], bf16, kind="ExternalInput")
    d_a1 = nc.dram_tensor("a1", [128, 512], bf16, kind="ExternalInput")
    d_wih1 = nc.dram_tensor("wih1t", [KT, 128, 512], bf16, kind="ExternalInput")
    d_whh1 = nc.dram_tensor("whh1t", [KT, 128, 512], bf16, kind="ExternalInput")
    d_xt = nc.dram_tensor("xt", [128, S], bf16, kind="ExternalInput")
    d_gb0 = nc.dram_tensor("gb0", [B, 512], bf16, kind="ExternalInput")
    d_gb1 = nc.dram_tensor("gb1", [B, 512], bf16, kind="ExternalInput")
    d_id = nc.dram_tensor("ident", [B, B], bf16, kind="ExternalInput")
    d_ench = nc.dram_tensor("ench", [128, KT * B], bf16, kind="ExternalInput")
    d_cinit = nc.dram_tensor("cinit", [B, 128], f32, kind="ExternalInput")
    d_linw8 = nc.dram_tensor("linw8", [KT, 128, VS], fp8, kind="ExternalInput")
    if use_linb:
        d_linb = nc.dram_tensor("linb", [1, VS], bf16, kind="ExternalInput")
    d_out = nc.dram_tensor("out", [S, VS], f32, kind="ExternalOutput")

    rg = [list(range(NCORES))]

    with tile.TileContext(nc) as tc, ExitStack() as ctx:
        wp = ctx.enter_context(tc.tile_pool(name="w", bufs=1))
        dp = ctx.enter_context(tc.tile_pool(name="db", bufs=6, space="DRAM"))
        hp = ctx.enter_context(tc.tile_pool(name="hx", bufs=3))
        cp = ctx.enter_context(tc.tile_pool(name="ct", bufs=2))
        tp = ctx.enter_context(tc.tile_pool(name="tmp", bufs=4))

        whh0 = wp.tile([128, KT * 512], bf16, name="whh0s")
        a1 = wp.tile([128, 512], bf16, name="a1s")
        wih1 = wp.tile([128, KT * 512], bf16, name="wih1s")
        whh1 = wp.tile([128, KT * 512], bf16, name="whh1s")
        xts = wp.tile([128, S], bf16, name="xts")
        gb0 = wp.tile([B, 512], bf16, name="gb0s")
        gb1 = wp.tile([B, 512], bf16, name="gb1s")
        idn = wp.tile([B, B], bf16, name="idns")
        ench = wp.tile([128, KT * B], bf16, name="enchs")
        h1store = wp.tile([128, KT * S], bf16, name="h1store")
        h8 = wp.tile([128, KT * S], fp8, name="h8store")
        linw8 = wp.tile([128, KT * VS], fp8, name="linw8s")
        cinit_sb = wp.tile([B, 128], f32, name="cinits")
        if use_linb:
            linb_sb = wp.tile([1, VS], bf16, name="linbs")
            ones = wp.tile([1, 128], bf16, name="ones")

        nc.sync.dma_start(
            whh0[:].rearrange("p (k g) -> p k g", k=KT),
            d_whh0[:].rearrange("k p g -> p k g"))
        nc.sync.dma_start(
            wih1[:].rearrange("p (k g) -> p k g", k=KT),
            d_wih1[:].rearrange("k p g -> p k g"))
        nc.sync.dma_start(
            whh1[:].rearrange("p (k g) -> p k g", k=KT),
            d_whh1[:].rearrange("k p g -> p k g"))
        nc.sync.dma_start(
            linw8[:].rearrange("p (k v) -> p k v", k=KT),
            d_linw8[:].rearrange("k p v -> p k v"))
        nc.sync.dma_start(a1[:], d_a1[:])
        nc.sync.dma_start(xts[:], d_xt[:])
        nc.sync.dma_start(gb0[:], d_gb0[:])
        nc.sync.dma_start(gb1[:], d_gb1[:])
        nc.sync.dma_start(idn[:], d_id[:])
        nc.sync.dma_start(ench[:], d_ench[:])
        nc.sync.dma_start(cinit_sb[:], d_cinit[:])
        if use_linb:
            nc.sync.dma_start(linb_sb[:], d_linb[:])
            nc.gpsimd.memset(ones[:], 1.0)

        whh0_k = whh0[:].rearrange("p (k g) -> p k g", k=KT)
        wih1_k = wih1[:].rearrange("p (k g) -> p k g", k=KT)
        whh1_k = whh1[:].rearrange("p (k g) -> p k g", k=KT)
        ench_k = ench[:].rearrange("p (k b) -> p k b", k=KT)
        h1s_k = h1store[:].rearrange("p (k s) -> p k s", k=KT)
        h8_k = h8[:].rearrange("p (k s) -> p k s", k=KT)
        lw_k = linw8[:].rearrange("p (k v) -> p k v", k=KT)

        for _rep in range(reps):
            ct0 = cp.tile([B, 128], f32, tag="ct0")
            ct1 = cp.tile([B, 128], f32, tag="ct1")
            nc.sync.dma_start(ct0[:], d_cinit[:])
            nc.sync.dma_start(ct1[:], d_cinit[:])

            with tc.tile_pool(name="pg", bufs=2, space="PSUM") as pg, \
                 tc.tile_pool(name="vp", bufs=4, space="PSUM") as vp, \
                 tc.tile_pool(name="lg", bufs=5) as lgp, \
                 tc.tile_pool(name="ob", bufs=2) as obp, \
                 tc.tile_pool(name="ex", bufs=2) as exp_p, \
                 tc.tile_pool(name="tot", bufs=1) as totp:

                totals = totp.tile([128, 16], f32, tag="totals")
                lse = totp.tile([128, 16], f32, tag="lse")
                neglse = totp.tile([128, 16], f32, tag="neglse")
                out_tb = d_out[:].rearrange("(b t) v -> t b v", b=B)
                lgt = {}

                def emit_vocab_m(m):
                    """fp8 DoubleRow vocab matmuls + exp for m-tile m."""
                    M = 128 if m < NMT - 1 else S - 128 * (NMT - 1)
                    msl = slice(m * 128, m * 128 + M)
                    lg = lgp.tile([128, VS], bf16, tag="lg")
                    lgt[m] = (lg, M)
                    for c in range(NCHK):
                        ps = vp.tile([128, CHUNK], f32, tag="ps")
                        for j in range(KT // 2):
                            nc.tensor.matmul(
                                ps[:M],
                                h8_k[:, 2 * j:2 * j + 2, msl],
                                lw_k[:, 2 * j:2 * j + 2,
                                     c * CHUNK:(c + 1) * CHUNK],
                                start=(j == 0),
                                stop=(not use_linb and j == KT // 2 - 1),
                                perf_mode=DR)
                        if use_linb:
                            nc.tensor.matmul(
                                ps[:M], ones[0:1, 0:M],
                                linb_sb[0:1, c * CHUNK:(c + 1) * CHUNK],
                                start=False, stop=True)
                        nc.vector.tensor_scalar_mul(
                            lg[:M, c * CHUNK:(c + 1) * CHUNK], ps[:M],
                            DESCALE)
                    ex = exp_p.tile([128, VS], bf16, tag="ex")
                    nc.scalar.activation(ex[:M], lg[:M, :], AF.Exp,
                                         accum_out=totals[:M, m:m + 1])

                def emit_ar(c0, c1):
                    """AllReduce exp-sums for m in [c0,c1), then outputs."""
                    nm = c1 - c0
                    ari = dp.tile([128, nm], f32, tag="ari")
                    aro = dp.tile([128, nm], f32, tag="aro",
                                  addr_space="Shared")
                    nc.sync.dma_start(ari[:], totals[:, c0:c1])
                    nc.gpsimd.collective_compute(
                        "AllReduce", ALU.add, replica_groups=rg,
                        ins=[ari[:].opt()], outs=[aro[:].opt()])
                    nc.sync.dma_start(lse[:, c0:c1], aro[:])
                    nc.scalar.activation(lse[:, c0:c1], lse[:, c0:c1], AF.Ln)
                    nc.vector.tensor_scalar_mul(neglse[:, c0:c1],
                                                lse[:, c0:c1], -1.0)
                    for m in range(c0, c1):
                        lg, M = lgt.pop(m)
                        q = M // 32
                        for h in range(2):
                            ob = obp.tile([128, VS // 2], f32, tag="ob")
                            if (m + h) % 2 == 0:
                                nc.vector.tensor_scalar(
                                    ob[:M],
                                    lg[:M, h * (VS // 2):(h + 1) * (VS // 2)],
                                    lse[:M, m:m + 1], None, op0=ALU.subtract)
                            else:
                                nc.scalar.activation(
                                    ob[:M],
                                    lg[:M, h * (VS // 2):(h + 1) * (VS // 2)],
                                    AF.Identity, bias=neglse[:M, m:m + 1])
                            dst = out_tb[m * 4:m * 4 + q, :,
                                         h * (VS // 2):(h + 1) * (VS // 2)]
                            nc.sync.dma_start(dst, ob[:M])

                hx0_m1 = ench_k       # h0(tau-1) tiles [128, k, 32]
                hx1_m2 = ench_k       # h1(tau-2) tiles

                # Layer 1 lags layer 0 by one tick: tick tau runs L0 for
                # step tau and L1 for step tau-1.  Two AllGathers per
                # tick: AG0 ships h0(tau) as soon as the L0 cell is done,
                # AG1 ships h1(tau-1) at the end.  whh1 matmuls are
                # emitted last so the previous tick's AG1 has landed.
                for tau in range(T + 1):
                    # ---- layer 0 matmuls for t = tau ----
                    if tau < T:
                        g0 = pg.tile([B, 512], f32, tag="g0")
                        nc.tensor.matmul(g0[:], idn[:], gb0[:],
                                         start=True, stop=False)
                        nc.tensor.matmul(g0[:], xts[:, tau * 32:(tau + 1) * 32],
                                         a1[:], start=False, stop=False)
                        for k in range(KT):
                            nc.tensor.matmul(g0[:], hx0_m1[:, k, :],
                                             whh0_k[:, k, :],
                                             start=False, stop=(k == KT - 1))
                        # L0 cell (batch-major [32, *])
                        sg0 = tp.tile([B, 384], bf16, tag="sg0")
                        nc.scalar.activation(sg0[:], g0[:, 0:384], AF.Sigmoid)
                        tg0 = tp.tile([B, 128], bf16, tag="tg0")
                        nc.scalar.activation(tg0[:], g0[:, 384:512], AF.Tanh)
                        t1 = tp.tile([B, 128], f32, tag="t1")
                        nc.vector.tensor_mul(t1[:], sg0[:, 0:128], tg0[:])
                        ct0n = cp.tile([B, 128], f32, tag="ct0")
                        nc.vector.tensor_mul(ct0n[:], sg0[:, 128:256], ct0[:])
                        nc.vector.tensor_add(ct0n[:], ct0n[:], t1[:])
                        ct0 = ct0n
                        th0 = tp.tile([B, 128], bf16, tag="th0")
                        nc.scalar.activation(th0[:], ct0[:], AF.Tanh)
                        h0b = tp.tile([B, 128], bf16, tag="h0b")
                        nc.vector.tensor_mul(h0b[:], sg0[:, 256:384], th0[:])
                        # [32,128] -> [128,32] exchange layout via 32x32
                        # block transpose + strided DMA
                        tb0 = tp.tile([B, 128], bf16, tag="tb0")
                        nc.vector.transpose(tb0[:], h0b[:])
                        bi0 = dp.tile([128, 32], bf16, tag="bi0")
                        bo0 = dp.tile([NCORES, 128, 32], bf16, tag="bo0",
                                      addr_space="Shared")
                        nc.sync.dma_start(
                            bi0[:].rearrange("(q j) i -> j q i", q=4),
                            tb0[:].rearrange("j (q i) -> j q i", q=4))
                        nc.gpsimd.collective_compute(
                            "AllGather", ALU.bypass, replica_groups=rg,
                            ins=[bi0[:].opt()], outs=[bo0[:].opt()])
                        hx0_new = hp.tile([128, KT * 32], bf16, tag="hx0")
                        nc.sync.dma_start(
                            hx0_new[:].rearrange("p (k c) -> p k c", k=KT),
                            bo0[:].rearrange("k p c -> p k c"))

                    # ---- layer 1 matmuls for t = tau-1 ----
                    if tau >= 1:
                        g1 = pg.tile([B, 512], f32, tag="g1")
                        nc.tensor.matmul(g1[:], idn[:], gb1[:],
                                         start=True, stop=False)
                        for k in range(KT):
                            nc.tensor.matmul(g1[:], hx0_m1[:, k, :],
                                             wih1_k[:, k, :],
                                             start=False, stop=False)
                        for k in range(KT):
                            nc.tensor.matmul(g1[:], hx1_m2[:, k, :],
                                             whh1_k[:, k, :],
                                             start=False, stop=(k == KT - 1))
                        sg1 = tp.tile([B, 384], bf16, tag="sg1")
                        nc.scalar.activation(sg1[:], g1[:, 0:384], AF.Sigmoid)
                        tg1 = tp.tile([B, 128], bf16, tag="tg1")
                        nc.scalar.activation(tg1[:], g1[:, 384:512], AF.Tanh)
                        t2 = tp.tile([B, 128], f32, tag="t2")
                        nc.vector.tensor_mul(t2[:], sg1[:, 0:128], tg1[:])
                        ct1n = cp.tile([B, 128], f32, tag="ct1")
                        nc.vector.tensor_mul(ct1n[:], sg1[:, 128:256], ct1[:])
                        nc.vector.tensor_add(ct1n[:], ct1n[:], t2[:])
                        ct1 = ct1n
                        th1 = tp.tile([B, 128], bf16, tag="th1")
                        nc.scalar.activation(th1[:], ct1[:], AF.Tanh)
                        h1b = tp.tile([B, 128], bf16, tag="h1b")
                        nc.vector.tensor_mul(h1b[:], sg1[:, 256:384], th1[:])
                        tb1 = tp.tile([B, 128], bf16, tag="tb1")
                        nc.vector.transpose(tb1[:], h1b[:])
                        bi1 = dp.tile([128, 32], bf16, tag="bi1")
                        bo1 = dp.tile([NCORES, 128, 32], bf16, tag="bo1",
                                      addr_space="Shared")
                        nc.sync.dma_start(
                            bi1[:].rearrange("(q j) i -> j q i", q=4),
                            tb1[:].rearrange("j (q i) -> j q i", q=4))
                        nc.gpsimd.collective_compute(
                            "AllGather", ALU.bypass, replica_groups=rg,
                            ins=[bi1[:].opt()], outs=[bo1[:].opt()])
                        if tau < T:
                            hx1_new = hp.tile([128, KT * 32], bf16, tag="hx1")
                            nc.sync.dma_start(
                                hx1_new[:].rearrange("p (k c) -> p k c", k=KT),
                                bo1[:].rearrange("k p c -> p k c"))
                            hx1_m2 = hx1_new[:].rearrange(
                                "p (k c) -> p k c", k=KT)
                        sl = slice((tau - 1) * 32, tau * 32)
                        nc.sync.dma_start(
                            h1s_k[:, :, sl], bo1[:].rearrange("k p c -> p k c"))
                        # off-critical-path fp8 cast for the vocab matmul
                        nc.vector.tensor_scalar_mul(
                            h8_k[:, :, sl], h1s_k[:, :, sl], SH)

                        # ---- interleaved vocab work for finished ticks ----
                        t1_ = tau - 1
                        if t1_ % 4 == 3 and t1_ // 4 < NMT - 1:
                            emit_vocab_m(t1_ // 4)
                            if t1_ == 15:
                                emit_ar(0, 4)
                            elif t1_ == 31:
                                emit_ar(4, 8)
                            elif t1_ == 47:
                                emit_ar(8, 12)

                    if tau < T:
                        hx0_m1 = hx0_new[:].rearrange("p (k c) -> p k c", k=KT)

                # tail: last m-tile (t=48,49) + its AllReduce + outputs
                emit_vocab_m(NMT - 1)
                emit_ar(12, 13)

    nc.compile()
    return nc


def _get_nc(reps=1, use_linb=False):
    key = ("nc", reps, use_linb)
    if key not in _BUILD_CACHE:
        _BUILD_CACHE[key] = _build(reps, use_linb)
    return _BUILD_CACHE[key]


def run(inputs, trace=False, reps=1):
    from concourse.bass_utils import run_bass_kernel_spmd

    in_maps, use_linb = _host_prep(inputs)
    nc = _get_nc(reps, use_linb)
    res = run_bass_kernel_spmd(nc, in_maps, core_ids=list(range(NCORES)),
                               trace=trace)
    full = np.empty((S, V), np.float32)
    for r in range(NCORES):
        full[:, r * VS:(r + 1) * VS] = res.results[r]["out"]
    return full, res


def kernel(**inputs):
    full, _ = run(inputs)
    return full
